# revision 2
# baseline (speedup 1.0000x reference)
"""Trainium2 Bass kernel for nn_BPFeedForward (per-element-type MLP, moe_routing).

Strategy: data-parallel over atoms. Each of the 8 cores gets 1/8 of each
element type's fingerprint rows (transposed to [D, m] on host), runs the
4-layer MLP with feature-on-partition / atoms-on-free layout:

    H_T[h, m] = tanh(W.T @ X_T + b)   (matmul lhsT = W as stored, rhs = X_T)

Matmul dtype is selectable: float32r (fp32 data, relaxed PE mode: full
speed at moving dim >= 256) or bfloat16. tanh+bias is fused into one
ScalarE activation per [128, n] tile reading straight from PSUM. The final
Wout layer produces [1, m] per element; DVE copies PSUM->SBUF, then DMA to
DRAM. Host gathers per-core [4, m] outputs and applies the reference's
segment-sum routing in numpy.
"""

import sys

if "/opt/trn_rl_repo" not in sys.path:
    sys.path.insert(0, "/opt/trn_rl_repo")

import numpy as np

N_CORES = 8
E = 4
N_ATOMS = 200000
M_TOTAL = N_ATOMS // E          # 50000 atoms per element type
MPC = M_TOTAL // N_CORES        # 6250 atoms per element per core
D = 128
H = 256
CHUNK = 512
MP = 6656                       # padded per-core atoms per element (13 x 512)
N_CHUNKS = MP // CHUNK

MODE = "f32r"                   # "f32r" or "bf16"

_COMPILED = {}


def _np_dtype(mode):
    if mode == "bf16":
        import ml_dtypes
        return ml_dtypes.bfloat16
    return np.float32


def _build_program(reps: int = 1, mode: str = MODE):
    import concourse.bass as bass  # noqa: F401
    import concourse.mybir as mybir
    import concourse.tile as tile
    from concourse import bacc

    F32 = mybir.dt.float32
    MMDT = mybir.dt.float32r if mode == "f32r" else mybir.dt.bfloat16
    Tanh = mybir.ActivationFunctionType.Tanh

    nc = bacc.Bacc(None, target_bir_lowering=False, debug=False)

    xt = nc.dram_tensor("xt", [E, D, MP], MMDT, kind="ExternalInput")
    w0 = nc.dram_tensor("w0", [128, E, H], MMDT, kind="ExternalInput")
    w1 = nc.dram_tensor("w1", [128, E, 2, H], MMDT, kind="ExternalInput")
    w2 = nc.dram_tensor("w2", [128, E, 2, H], MMDT, kind="ExternalInput")
    wo = nc.dram_tensor("wo", [128, E, 2], MMDT, kind="ExternalInput")
    b0 = nc.dram_tensor("b0", [128, E, 2], F32, kind="ExternalInput")
    b1 = nc.dram_tensor("b1", [128, E, 2], F32, kind="ExternalInput")
    b2 = nc.dram_tensor("b2", [128, E, 2], F32, kind="ExternalInput")
    out = nc.dram_tensor("out", [E, MP], F32, kind="ExternalOutput")

    with tile.TileContext(nc) as tc:
        with (
            tc.tile_pool(name="consts", bufs=1) as consts,
            tc.tile_pool(name="xin", bufs=4) as xin,
            tc.tile_pool(name="acts", bufs=6) as actp,
            tc.tile_pool(name="osb", bufs=4) as osbp,
            tc.tile_pool(name="psum", bufs=3, space="PSUM") as psp,
            tc.tile_pool(name="psout", bufs=2, space="PSUM") as psop,
        ):
            w0_t = consts.tile([128, E, H], MMDT)
            nc.sync.dma_start(out=w0_t[:], in_=w0[:])
            w1_t = consts.tile([128, E, 2, H], MMDT)
            nc.sync.dma_start(out=w1_t[:], in_=w1[:])
            w2_t = consts.tile([128, E, 2, H], MMDT)
            nc.sync.dma_start(out=w2_t[:], in_=w2[:])
            wo_t = consts.tile([128, E, 2], MMDT)
            nc.sync.dma_start(out=wo_t[:], in_=wo[:])
            b0_t = consts.tile([128, E, 2], F32)
            nc.sync.dma_start(out=b0_t[:], in_=b0[:])
            b1_t = consts.tile([128, E, 2], F32)
            nc.sync.dma_start(out=b1_t[:], in_=b1[:])
            b2_t = consts.tile([128, E, 2], F32)
            nc.sync.dma_start(out=b2_t[:], in_=b2[:])

            for _rep in range(reps):
                for e in range(E):
                    for j in range(N_CHUNKS):
                        n = CHUNK
                        c0 = j * CHUNK

                        x = xin.tile([128, CHUNK], MMDT, tag="x")
                        nc.sync.dma_start(out=x[:, :n], in_=xt[e, :, c0:c0 + n])

                        # ---- layer 0: [D=128] -> [H=256] ----
                        ps0 = psp.tile([128, 2 * CHUNK], F32, tag="ps")
                        for ht in range(2):
                            nc.tensor.matmul(
                                ps0[:, ht * CHUNK: ht * CHUNK + n],
                                w0_t[:, e, ht * 128:(ht + 1) * 128],
                                x[:, :n],
                            )
                        a0 = actp.tile([128, 2 * CHUNK], MMDT, tag="a")
                        for ht in range(2):
                            nc.scalar.activation(
                                out=a0[:, ht * CHUNK: ht * CHUNK + n],
                                in_=ps0[:, ht * CHUNK: ht * CHUNK + n],
                                func=Tanh,
                                bias=b0_t[:, e, ht:ht + 1],
                                scale=1.0,
                            )

                        # ---- layers 1, 2: [256] -> [256] ----
                        prev = a0
                        for w_t, b_t in ((w1_t, b1_t), (w2_t, b2_t)):
                            ps = psp.tile([128, 2 * CHUNK], F32, tag="ps")
                            for ht in range(2):
                                for kt in range(2):
                                    nc.tensor.matmul(
                                        ps[:, ht * CHUNK: ht * CHUNK + n],
                                        w_t[:, e, kt, ht * 128:(ht + 1) * 128],
                                        prev[:, kt * CHUNK: kt * CHUNK + n],
                                        start=(kt == 0),
                                        stop=(kt == 1),
                                    )
                            a = actp.tile([128, 2 * CHUNK], MMDT, tag="a")
                            for ht in range(2):
                                nc.scalar.activation(
                                    out=a[:, ht * CHUNK: ht * CHUNK + n],
                                    in_=ps[:, ht * CHUNK: ht * CHUNK + n],
                                    func=Tanh,
                                    bias=b_t[:, e, ht:ht + 1],
                                    scale=1.0,
                                )
                            prev = a

                        # ---- output layer: [256] -> [1], no bias/act ----
                        pso = psop.tile([128, CHUNK], F32, tag="o")
                        for kt in range(2):
                            nc.tensor.matmul(
                                pso[:1, :n],
                                wo_t[:, e, kt:kt + 1],
                                prev[:, kt * CHUNK: kt * CHUNK + n],
                                start=(kt == 0),
                                stop=(kt == 1),
                            )
                        o_sb = osbp.tile([1, CHUNK], F32, tag="osb")
                        nc.vector.tensor_copy(out=o_sb[:1, :n], in_=pso[:1, :n])
                        nc.sync.dma_start(out=out[e:e + 1, c0:c0 + n], in_=o_sb[:1, :n])

    nc.compile()
    return nc


def _get_compiled(mode=MODE):
    if mode not in _COMPILED:
        _COMPILED[mode] = _build_program(reps=1, mode=mode)
    return _COMPILED[mode]


def _prep_core_inputs(fps, W0, b0, W1, b1, W2, b2, Wout, mode=MODE):
    """Host-side shard + layout prep. Returns list of per-core input dicts."""
    f32 = np.float32
    mdt = _np_dtype(mode)

    def cvt(a):
        return np.ascontiguousarray(a).astype(mdt, copy=False)

    w0_dev = cvt(np.transpose(W0, (1, 0, 2)))
    w1_dev = cvt(W1.reshape(E, 2, 128, H).transpose(2, 0, 1, 3))
    w2_dev = cvt(W2.reshape(E, 2, 128, H).transpose(2, 0, 1, 3))
    wo_dev = cvt(Wout.reshape(E, 2, 128).transpose(2, 0, 1))
    b0_dev = np.ascontiguousarray(b0.reshape(E, 2, 128).transpose(2, 0, 1)).astype(f32)
    b1_dev = np.ascontiguousarray(b1.reshape(E, 2, 128).transpose(2, 0, 1)).astype(f32)
    b2_dev = np.ascontiguousarray(b2.reshape(E, 2, 128).transpose(2, 0, 1)).astype(f32)

    in_maps = []
    for c in range(N_CORES):
        xtc = np.zeros((E, D, MP), mdt)
        for e in range(E):
            xtc[e, :, :MPC] = fps[e][c * MPC:(c + 1) * MPC].T.astype(mdt, copy=False)
        in_maps.append({
            "xt": xtc,
            "w0": w0_dev, "w1": w1_dev, "w2": w2_dev, "wo": wo_dev,
            "b0": b0_dev, "b1": b1_dev, "b2": b2_dev,
        })
    return in_maps


def _route_outputs(flat_per_elem, elems, n_atoms):
    """Replicate reference routing: idx = concat(nonzero(elems==e, size=M))
    then segment_sum. nonzero(size=M) truncates or zero-pads."""
    out = np.zeros((n_atoms,), np.float32)
    for e in range(E):
        idx_e = np.nonzero(elems == e)[0]
        if idx_e.shape[0] >= M_TOTAL:
            idx_e = idx_e[:M_TOTAL]
        else:
            idx_e = np.concatenate(
                [idx_e, np.zeros(M_TOTAL - idx_e.shape[0], idx_e.dtype)])
        np.add.at(out, idx_e, flat_per_elem[e])
    return out


def kernel(fps_0, fps_1, fps_2, fps_3, W0, b0, W1, b1, W2, b2, Wout,
           elems, ind_1):
    from concourse.bass_utils import run_bass_kernel_spmd

    f32 = np.float32
    fps = [np.asarray(f, dtype=f32) for f in (fps_0, fps_1, fps_2, fps_3)]
    W0 = np.asarray(W0, dtype=f32)
    W1 = np.asarray(W1, dtype=f32)
    W2 = np.asarray(W2, dtype=f32)
    Wout = np.asarray(Wout, dtype=f32)
    b0 = np.asarray(b0, dtype=f32)
    b1 = np.asarray(b1, dtype=f32)
    b2 = np.asarray(b2, dtype=f32)
    elems = np.asarray(elems)
    n_atoms = np.asarray(ind_1).shape[0]

    nc = _get_compiled()
    in_maps = _prep_core_inputs(fps, W0, b0, W1, b1, W2, b2, Wout)
    res = run_bass_kernel_spmd(nc, in_maps, core_ids=list(range(N_CORES)))

    # [E, M_TOTAL] in element-major order (same as reference's out_e)
    flat = np.empty((E, M_TOTAL), f32)
    for c in range(N_CORES):
        o = res.results[c]["out"]          # [E, MP]
        flat[:, c * MPC:(c + 1) * MPC] = o[:, :MPC]

    out = _route_outputs(flat, elems, n_atoms)
    return out.reshape(n_atoms, 1).astype(f32)


# revision 4
# speedup vs baseline: 1.8783x; 1.8783x over previous
"""Trainium2 Bass kernel for nn_BPFeedForward (per-element-type MLP, moe_routing).

Strategy: data-parallel over atoms. Each of the 8 cores gets 1/8 of each
element type's fingerprint rows (transposed to [D, m] on host), runs the
4-layer MLP with feature-on-partition / atoms-on-free layout:

    H_T[h, m] = tanh(W.T @ X_T + b)   (matmul lhsT = W as stored, rhs = X_T)

Matmul dtype is selectable: float32r (fp32 data, relaxed PE mode: full
speed at moving dim >= 256) or bfloat16. tanh+bias is fused into one
ScalarE activation per [128, n] tile reading straight from PSUM. The final
Wout layer produces [1, m] per element; DVE copies PSUM->SBUF, then DMA to
DRAM. Host gathers per-core [4, m] outputs and applies the reference's
segment-sum routing in numpy.
"""

import sys

if "/opt/trn_rl_repo" not in sys.path:
    sys.path.insert(0, "/opt/trn_rl_repo")

import numpy as np

N_CORES = 8
E = 4
N_ATOMS = 200000
M_TOTAL = N_ATOMS // E          # 50000 atoms per element type
MPC = M_TOTAL // N_CORES        # 6250 atoms per element per core
D = 128
H = 256
CHUNK = 512
SUPER = 1024                    # superchunk width (ACT batch unit)
MP = 6656                       # padded per-core atoms per element (13 x 512)
N_CHUNKS = MP // CHUNK

MODE = "f32r"                   # "f32r" or "bf16"

_COMPILED = {}


def _np_dtype(mode):
    if mode == "bf16":
        import ml_dtypes
        return ml_dtypes.bfloat16
    return np.float32


def _build_program(reps: int = 1, mode: str = MODE):
    import concourse.bass as bass  # noqa: F401
    import concourse.mybir as mybir
    import concourse.tile as tile
    from concourse import bacc

    F32 = mybir.dt.float32
    MMDT = mybir.dt.float32r if mode == "f32r" else mybir.dt.bfloat16
    Tanh = mybir.ActivationFunctionType.Tanh

    nc = bacc.Bacc(None, target_bir_lowering=False, debug=False)

    xt = nc.dram_tensor("xt", [E, D, MP], MMDT, kind="ExternalInput")
    w0 = nc.dram_tensor("w0", [128, E, H], MMDT, kind="ExternalInput")
    w1 = nc.dram_tensor("w1", [128, E, 2, H], MMDT, kind="ExternalInput")
    w2 = nc.dram_tensor("w2", [128, E, 2, H], MMDT, kind="ExternalInput")
    wo = nc.dram_tensor("wo", [128, E, 2], MMDT, kind="ExternalInput")
    b0 = nc.dram_tensor("b0", [128, E, 2], F32, kind="ExternalInput")
    b1 = nc.dram_tensor("b1", [128, E, 2], F32, kind="ExternalInput")
    b2 = nc.dram_tensor("b2", [128, E, 2], F32, kind="ExternalInput")
    out = nc.dram_tensor("out", [E, MP], F32, kind="ExternalOutput")

    # superchunk spans per element: [start, width] with width 1024 or 512
    spans = []
    pos = 0
    while pos < MP:
        w = min(SUPER, MP - pos)
        spans.append((pos, w))
        pos += w

    with tile.TileContext(nc) as tc:
        with (
            tc.tile_pool(name="consts", bufs=1) as consts,
            tc.tile_pool(name="xin", bufs=4) as xin,
            tc.tile_pool(name="acts", bufs=6) as actp,
            tc.tile_pool(name="osb", bufs=4) as osbp,
            tc.tile_pool(name="psum", bufs=4, space="PSUM") as psp,
        ):
            w0_t = consts.tile([128, E, H], MMDT)
            nc.sync.dma_start(out=w0_t[:], in_=w0[:])
            w1_t = consts.tile([128, E, 2, H], MMDT)
            nc.sync.dma_start(out=w1_t[:], in_=w1[:])
            w2_t = consts.tile([128, E, 2, H], MMDT)
            nc.sync.dma_start(out=w2_t[:], in_=w2[:])
            wo_t = consts.tile([128, E, 2], MMDT)
            nc.sync.dma_start(out=wo_t[:], in_=wo[:])
            b0_t = consts.tile([128, E, 2], F32)
            nc.sync.dma_start(out=b0_t[:], in_=b0[:])
            b1_t = consts.tile([128, E, 2], F32)
            nc.sync.dma_start(out=b1_t[:], in_=b1[:])
            b2_t = consts.tile([128, E, 2], F32)
            nc.sync.dma_start(out=b2_t[:], in_=b2[:])

            for _rep in range(reps):
                for e in range(E):
                    for c0, w in spans:
                        nch = w // CHUNK       # 512-wide matmul columns

                        x = xin.tile([128, SUPER], MMDT, tag="x")
                        nc.sync.dma_start(out=x[:, :w], in_=xt[e, :, c0:c0 + w])

                        # ---- layer 0: [D=128] -> [H=256] ----
                        # one psum tile per h-tile, [128, w]; one ACT per h-tile
                        a0 = actp.tile([128, 2 * SUPER], MMDT, tag="a")
                        for ht in range(2):
                            ps = psp.tile([128, SUPER], F32, tag="ps")
                            for ci in range(nch):
                                cs = ci * CHUNK
                                nc.tensor.matmul(
                                    ps[:, cs:cs + CHUNK],
                                    w0_t[:, e, ht * 128:(ht + 1) * 128],
                                    x[:, cs:cs + CHUNK],
                                )
                            nc.scalar.activation(
                                out=a0[:, ht * w: ht * w + w],
                                in_=ps[:, :w],
                                func=Tanh,
                                bias=b0_t[:, e, ht:ht + 1],
                                scale=1.0,
                            )

                        # ---- layers 1, 2: [256] -> [256] ----
                        prev = a0
                        for w_t, b_t in ((w1_t, b1_t), (w2_t, b2_t)):
                            a = actp.tile([128, 2 * SUPER], MMDT, tag="a")
                            for ht in range(2):
                                ps = psp.tile([128, SUPER], F32, tag="ps")
                                for ci in range(nch):
                                    cs = ci * CHUNK
                                    for kt in range(2):
                                        nc.tensor.matmul(
                                            ps[:, cs:cs + CHUNK],
                                            w_t[:, e, kt, ht * 128:(ht + 1) * 128],
                                            prev[:, kt * w + cs: kt * w + cs + CHUNK],
                                            start=(kt == 0),
                                            stop=(kt == 1),
                                        )
                                nc.scalar.activation(
                                    out=a[:, ht * w: ht * w + w],
                                    in_=ps[:, :w],
                                    func=Tanh,
                                    bias=b_t[:, e, ht:ht + 1],
                                    scale=1.0,
                                )
                            prev = a

                        # ---- output layer: [256] -> [1], no bias/act ----
                        pso = psp.tile([128, SUPER], F32, tag="ps")
                        for ci in range(nch):
                            cs = ci * CHUNK
                            for kt in range(2):
                                nc.tensor.matmul(
                                    pso[:1, cs:cs + CHUNK],
                                    wo_t[:, e, kt:kt + 1],
                                    prev[:, kt * w + cs: kt * w + cs + CHUNK],
                                    start=(kt == 0),
                                    stop=(kt == 1),
                                )
                        o_sb = osbp.tile([1, SUPER], F32, tag="osb")
                        nc.vector.tensor_copy(out=o_sb[:1, :w], in_=pso[:1, :w])
                        nc.sync.dma_start(out=out[e:e + 1, c0:c0 + w], in_=o_sb[:1, :w])

    nc.compile()
    return nc


def _get_compiled(mode=MODE):
    if mode not in _COMPILED:
        _COMPILED[mode] = _build_program(reps=1, mode=mode)
    return _COMPILED[mode]


def _prep_core_inputs(fps, W0, b0, W1, b1, W2, b2, Wout, mode=MODE):
    """Host-side shard + layout prep. Returns list of per-core input dicts."""
    f32 = np.float32
    mdt = _np_dtype(mode)

    def cvt(a):
        return np.ascontiguousarray(a).astype(mdt, copy=False)

    w0_dev = cvt(np.transpose(W0, (1, 0, 2)))
    w1_dev = cvt(W1.reshape(E, 2, 128, H).transpose(2, 0, 1, 3))
    w2_dev = cvt(W2.reshape(E, 2, 128, H).transpose(2, 0, 1, 3))
    wo_dev = cvt(Wout.reshape(E, 2, 128).transpose(2, 0, 1))
    b0_dev = np.ascontiguousarray(b0.reshape(E, 2, 128).transpose(2, 0, 1)).astype(f32)
    b1_dev = np.ascontiguousarray(b1.reshape(E, 2, 128).transpose(2, 0, 1)).astype(f32)
    b2_dev = np.ascontiguousarray(b2.reshape(E, 2, 128).transpose(2, 0, 1)).astype(f32)

    in_maps = []
    for c in range(N_CORES):
        xtc = np.zeros((E, D, MP), mdt)
        for e in range(E):
            xtc[e, :, :MPC] = fps[e][c * MPC:(c + 1) * MPC].T.astype(mdt, copy=False)
        in_maps.append({
            "xt": xtc,
            "w0": w0_dev, "w1": w1_dev, "w2": w2_dev, "wo": wo_dev,
            "b0": b0_dev, "b1": b1_dev, "b2": b2_dev,
        })
    return in_maps


def _route_outputs(flat_per_elem, elems, n_atoms):
    """Replicate reference routing: idx = concat(nonzero(elems==e, size=M))
    then segment_sum. nonzero(size=M) truncates or zero-pads."""
    out = np.zeros((n_atoms,), np.float32)
    for e in range(E):
        idx_e = np.nonzero(elems == e)[0]
        if idx_e.shape[0] >= M_TOTAL:
            idx_e = idx_e[:M_TOTAL]
        else:
            idx_e = np.concatenate(
                [idx_e, np.zeros(M_TOTAL - idx_e.shape[0], idx_e.dtype)])
        np.add.at(out, idx_e, flat_per_elem[e])
    return out


def kernel(fps_0, fps_1, fps_2, fps_3, W0, b0, W1, b1, W2, b2, Wout,
           elems, ind_1):
    from concourse.bass_utils import run_bass_kernel_spmd

    f32 = np.float32
    fps = [np.asarray(f, dtype=f32) for f in (fps_0, fps_1, fps_2, fps_3)]
    W0 = np.asarray(W0, dtype=f32)
    W1 = np.asarray(W1, dtype=f32)
    W2 = np.asarray(W2, dtype=f32)
    Wout = np.asarray(Wout, dtype=f32)
    b0 = np.asarray(b0, dtype=f32)
    b1 = np.asarray(b1, dtype=f32)
    b2 = np.asarray(b2, dtype=f32)
    elems = np.asarray(elems)
    n_atoms = np.asarray(ind_1).shape[0]

    nc = _get_compiled()
    in_maps = _prep_core_inputs(fps, W0, b0, W1, b1, W2, b2, Wout)
    res = run_bass_kernel_spmd(nc, in_maps, core_ids=list(range(N_CORES)))

    # [E, M_TOTAL] in element-major order (same as reference's out_e)
    flat = np.empty((E, M_TOTAL), f32)
    for c in range(N_CORES):
        o = res.results[c]["out"]          # [E, MP]
        flat[:, c * MPC:(c + 1) * MPC] = o[:, :MPC]

    out = _route_outputs(flat, elems, n_atoms)
    return out.reshape(n_atoms, 1).astype(f32)


# revision 5
# speedup vs baseline: 1.9822x; 1.0553x over previous
"""Trainium2 Bass kernel for nn_BPFeedForward (per-element-type MLP, moe_routing).

Strategy: data-parallel over atoms. Each of the 8 cores gets 1/8 of each
element type's fingerprint rows (transposed to [D, m] on host), runs the
4-layer MLP with feature-on-partition / atoms-on-free layout:

    H_T[h, m] = tanh(W.T @ X_T + b)   (matmul lhsT = W as stored, rhs = X_T)

Matmul dtype is selectable: float32r (fp32 data, relaxed PE mode: full
speed at moving dim >= 256) or bfloat16. tanh+bias is fused into one
ScalarE activation per [128, n] tile reading straight from PSUM. The final
Wout layer produces [1, m] per element; DVE copies PSUM->SBUF, then DMA to
DRAM. Host gathers per-core [4, m] outputs and applies the reference's
segment-sum routing in numpy.
"""

import sys

if "/opt/trn_rl_repo" not in sys.path:
    sys.path.insert(0, "/opt/trn_rl_repo")

import numpy as np

N_CORES = 8
E = 4
N_ATOMS = 200000
M_TOTAL = N_ATOMS // E          # 50000 atoms per element type
MPC = M_TOTAL // N_CORES        # 6250 atoms per element per core
D = 128
H = 256
CHUNK = 512
SUPER = 1024                    # superchunk width (ACT batch unit)
MP = MPC                        # per-core atoms per element (no padding)

MODE = "f32r"                   # "f32r" or "bf16"

_COMPILED = {}


def _np_dtype(mode):
    if mode == "bf16":
        import ml_dtypes
        return ml_dtypes.bfloat16
    return np.float32


def _build_program(reps: int = 1, mode: str = MODE):
    import concourse.bass as bass  # noqa: F401
    import concourse.mybir as mybir
    import concourse.tile as tile
    from concourse import bacc

    F32 = mybir.dt.float32
    MMDT = mybir.dt.float32r if mode == "f32r" else mybir.dt.bfloat16
    Tanh = mybir.ActivationFunctionType.Tanh

    nc = bacc.Bacc(None, target_bir_lowering=False, debug=False)

    xt = nc.dram_tensor("xt", [E, D, MP], MMDT, kind="ExternalInput")
    w0 = nc.dram_tensor("w0", [128, E, H], MMDT, kind="ExternalInput")
    w1 = nc.dram_tensor("w1", [128, E, 2, H], MMDT, kind="ExternalInput")
    w2 = nc.dram_tensor("w2", [128, E, 2, H], MMDT, kind="ExternalInput")
    wo = nc.dram_tensor("wo", [128, E, 2], MMDT, kind="ExternalInput")
    b0 = nc.dram_tensor("b0", [128, E, 2], F32, kind="ExternalInput")
    b1 = nc.dram_tensor("b1", [128, E, 2], F32, kind="ExternalInput")
    b2 = nc.dram_tensor("b2", [128, E, 2], F32, kind="ExternalInput")
    out = nc.dram_tensor("out", [E, MP], F32, kind="ExternalOutput")

    # superchunk spans per element: [start, width] with width 1024 or 512
    spans = []
    pos = 0
    while pos < MP:
        w = min(SUPER, MP - pos)
        spans.append((pos, w))
        pos += w

    with tile.TileContext(nc) as tc:
        with (
            tc.tile_pool(name="consts", bufs=1) as consts,
            tc.tile_pool(name="xin", bufs=6) as xin,
            tc.tile_pool(name="acts", bufs=8) as actp,
            tc.tile_pool(name="osb", bufs=4) as osbp,
            tc.tile_pool(name="psum", bufs=4, space="PSUM") as psp,
        ):
            w0_t = consts.tile([128, E, H], MMDT)
            nc.sync.dma_start(out=w0_t[:], in_=w0[:])
            w1_t = consts.tile([128, E, 2, H], MMDT)
            nc.sync.dma_start(out=w1_t[:], in_=w1[:])
            w2_t = consts.tile([128, E, 2, H], MMDT)
            nc.sync.dma_start(out=w2_t[:], in_=w2[:])
            wo_t = consts.tile([128, E, 2], MMDT)
            nc.sync.dma_start(out=wo_t[:], in_=wo[:])
            b0_t = consts.tile([128, E, 2], F32)
            nc.sync.dma_start(out=b0_t[:], in_=b0[:])
            b1_t = consts.tile([128, E, 2], F32)
            nc.sync.dma_start(out=b1_t[:], in_=b1[:])
            b2_t = consts.tile([128, E, 2], F32)
            nc.sync.dma_start(out=b2_t[:], in_=b2[:])

            for _rep in range(reps):
                for e in range(E):
                    for c0, w in spans:
                        x = xin.tile([128, SUPER], MMDT, tag="x")
                        nc.sync.dma_start(out=x[:, :w], in_=xt[e, :, c0:c0 + w])
                        cols = [(cs, min(CHUNK, w - cs))
                                for cs in range(0, w, CHUNK)]

                        # ---- layer 0: [D=128] -> [H=256] ----
                        # one psum tile per h-tile, [128, w]; one ACT per h-tile
                        a0 = actp.tile([128, 2 * SUPER], MMDT, tag="a")
                        for ht in range(2):
                            ps = psp.tile([128, SUPER], F32, tag="ps")
                            for cs, cw in cols:
                                nc.tensor.matmul(
                                    ps[:, cs:cs + cw],
                                    w0_t[:, e, ht * 128:(ht + 1) * 128],
                                    x[:, cs:cs + cw],
                                )
                            nc.scalar.activation(
                                out=a0[:, ht * w: ht * w + w],
                                in_=ps[:, :w],
                                func=Tanh,
                                bias=b0_t[:, e, ht:ht + 1],
                                scale=1.0,
                            )

                        # ---- layers 1, 2: [256] -> [256] ----
                        prev = a0
                        for w_t, b_t in ((w1_t, b1_t), (w2_t, b2_t)):
                            a = actp.tile([128, 2 * SUPER], MMDT, tag="a")
                            for ht in range(2):
                                ps = psp.tile([128, SUPER], F32, tag="ps")
                                for cs, cw in cols:
                                    for kt in range(2):
                                        nc.tensor.matmul(
                                            ps[:, cs:cs + cw],
                                            w_t[:, e, kt, ht * 128:(ht + 1) * 128],
                                            prev[:, kt * w + cs: kt * w + cs + cw],
                                            start=(kt == 0),
                                            stop=(kt == 1),
                                        )
                                nc.scalar.activation(
                                    out=a[:, ht * w: ht * w + w],
                                    in_=ps[:, :w],
                                    func=Tanh,
                                    bias=b_t[:, e, ht:ht + 1],
                                    scale=1.0,
                                )
                            prev = a

                        # ---- output layer: [256] -> [1], no bias/act ----
                        pso = psp.tile([128, SUPER], F32, tag="ps")
                        for cs, cw in cols:
                            for kt in range(2):
                                nc.tensor.matmul(
                                    pso[:1, cs:cs + cw],
                                    wo_t[:, e, kt:kt + 1],
                                    prev[:, kt * w + cs: kt * w + cs + cw],
                                    start=(kt == 0),
                                    stop=(kt == 1),
                                )
                        o_sb = osbp.tile([1, SUPER], F32, tag="osb")
                        nc.vector.tensor_copy(out=o_sb[:1, :w], in_=pso[:1, :w])
                        nc.sync.dma_start(out=out[e:e + 1, c0:c0 + w], in_=o_sb[:1, :w])

    nc.compile()
    return nc


def _get_compiled(mode=MODE):
    if mode not in _COMPILED:
        _COMPILED[mode] = _build_program(reps=1, mode=mode)
    return _COMPILED[mode]


def _prep_core_inputs(fps, W0, b0, W1, b1, W2, b2, Wout, mode=MODE):
    """Host-side shard + layout prep. Returns list of per-core input dicts."""
    f32 = np.float32
    mdt = _np_dtype(mode)

    def cvt(a):
        return np.ascontiguousarray(a).astype(mdt, copy=False)

    w0_dev = cvt(np.transpose(W0, (1, 0, 2)))
    w1_dev = cvt(W1.reshape(E, 2, 128, H).transpose(2, 0, 1, 3))
    w2_dev = cvt(W2.reshape(E, 2, 128, H).transpose(2, 0, 1, 3))
    wo_dev = cvt(Wout.reshape(E, 2, 128).transpose(2, 0, 1))
    b0_dev = np.ascontiguousarray(b0.reshape(E, 2, 128).transpose(2, 0, 1)).astype(f32)
    b1_dev = np.ascontiguousarray(b1.reshape(E, 2, 128).transpose(2, 0, 1)).astype(f32)
    b2_dev = np.ascontiguousarray(b2.reshape(E, 2, 128).transpose(2, 0, 1)).astype(f32)

    in_maps = []
    for c in range(N_CORES):
        xtc = np.zeros((E, D, MP), mdt)
        for e in range(E):
            xtc[e, :, :MPC] = fps[e][c * MPC:(c + 1) * MPC].T.astype(mdt, copy=False)
        in_maps.append({
            "xt": xtc,
            "w0": w0_dev, "w1": w1_dev, "w2": w2_dev, "wo": wo_dev,
            "b0": b0_dev, "b1": b1_dev, "b2": b2_dev,
        })
    return in_maps


def _route_outputs(flat_per_elem, elems, n_atoms):
    """Replicate reference routing: idx = concat(nonzero(elems==e, size=M))
    then segment_sum. nonzero(size=M) truncates or zero-pads."""
    out = np.zeros((n_atoms,), np.float32)
    for e in range(E):
        idx_e = np.nonzero(elems == e)[0]
        if idx_e.shape[0] >= M_TOTAL:
            idx_e = idx_e[:M_TOTAL]
        else:
            idx_e = np.concatenate(
                [idx_e, np.zeros(M_TOTAL - idx_e.shape[0], idx_e.dtype)])
        np.add.at(out, idx_e, flat_per_elem[e])
    return out


def kernel(fps_0, fps_1, fps_2, fps_3, W0, b0, W1, b1, W2, b2, Wout,
           elems, ind_1):
    from concourse.bass_utils import run_bass_kernel_spmd

    f32 = np.float32
    fps = [np.asarray(f, dtype=f32) for f in (fps_0, fps_1, fps_2, fps_3)]
    W0 = np.asarray(W0, dtype=f32)
    W1 = np.asarray(W1, dtype=f32)
    W2 = np.asarray(W2, dtype=f32)
    Wout = np.asarray(Wout, dtype=f32)
    b0 = np.asarray(b0, dtype=f32)
    b1 = np.asarray(b1, dtype=f32)
    b2 = np.asarray(b2, dtype=f32)
    elems = np.asarray(elems)
    n_atoms = np.asarray(ind_1).shape[0]

    nc = _get_compiled()
    in_maps = _prep_core_inputs(fps, W0, b0, W1, b1, W2, b2, Wout)
    res = run_bass_kernel_spmd(nc, in_maps, core_ids=list(range(N_CORES)))

    # [E, M_TOTAL] in element-major order (same as reference's out_e)
    flat = np.empty((E, M_TOTAL), f32)
    for c in range(N_CORES):
        o = res.results[c]["out"]          # [E, MP]
        flat[:, c * MPC:(c + 1) * MPC] = o[:, :MPC]

    out = _route_outputs(flat, elems, n_atoms)
    return out.reshape(n_atoms, 1).astype(f32)


# revision 7
# speedup vs baseline: 3.5653x; 1.7987x over previous
"""Trainium2 Bass kernel for nn_BPFeedForward (per-element-type MLP, moe_routing).

Strategy: data-parallel over atoms. Each of the 8 cores gets 1/8 of each
element type's fingerprint rows (transposed to [D, m] on host), runs the
4-layer MLP with feature-on-partition / atoms-on-free layout:

    H_T[h, m] = tanh(W.T @ X_T + b)   (matmul lhsT = W as stored, rhs = X_T)

Matmul dtype is selectable: float32r (fp32 data, relaxed PE mode: full
speed at moving dim >= 256) or bfloat16. tanh+bias is fused into one
ScalarE activation per [128, n] tile reading straight from PSUM. The final
Wout layer produces [1, m] per element; DVE copies PSUM->SBUF, then DMA to
DRAM. Host gathers per-core [4, m] outputs and applies the reference's
segment-sum routing in numpy.
"""

import sys

if "/opt/trn_rl_repo" not in sys.path:
    sys.path.insert(0, "/opt/trn_rl_repo")

import numpy as np

N_CORES = 8
E = 4
N_ATOMS = 200000
M_TOTAL = N_ATOMS // E          # 50000 atoms per element type
MPC = M_TOTAL // N_CORES        # 6250 atoms per element per core
D = 128
H = 256
CHUNK = 512
SUPER = 1024                    # superchunk width (ACT batch unit)
MP = MPC                        # per-core atoms per element (no padding)

MODE = "f32r"                   # "f32r" or "bf16"

_COMPILED = {}


def _np_dtype(mode):
    if mode == "bf16":
        import ml_dtypes
        return ml_dtypes.bfloat16
    return np.float32


def _build_program(reps: int = 1, mode: str = MODE):
    import concourse.bass as bass  # noqa: F401
    import concourse.mybir as mybir
    import concourse.tile as tile
    from concourse import bacc

    F32 = mybir.dt.float32
    MMDT = mybir.dt.float32r if mode == "f32r" else mybir.dt.bfloat16
    Tanh = mybir.ActivationFunctionType.Tanh

    nc = bacc.Bacc(None, target_bir_lowering=False, debug=False)

    xt = nc.dram_tensor("xt", [E, D, MP], MMDT, kind="ExternalInput")
    w0 = nc.dram_tensor("w0", [128, E, H], MMDT, kind="ExternalInput")
    w1 = nc.dram_tensor("w1", [128, E, 2, H], MMDT, kind="ExternalInput")
    w2 = nc.dram_tensor("w2", [128, E, 2, H], MMDT, kind="ExternalInput")
    wo = nc.dram_tensor("wo", [128, E, 2], MMDT, kind="ExternalInput")
    b0 = nc.dram_tensor("b0", [128, E, 2], F32, kind="ExternalInput")
    b1 = nc.dram_tensor("b1", [128, E, 2], F32, kind="ExternalInput")
    b2 = nc.dram_tensor("b2", [128, E, 2], F32, kind="ExternalInput")
    out = nc.dram_tensor("out", [E, MP], F32, kind="ExternalOutput")

    # superchunk spans per element: [start, width] with width 1024 or 512
    spans = []
    pos = 0
    while pos < MP:
        w = min(SUPER, MP - pos)
        spans.append((pos, w))
        pos += w

    with tile.TileContext(nc) as tc:
        with (
            tc.tile_pool(name="consts", bufs=1) as consts,
            tc.tile_pool(name="xin", bufs=6) as xin,
            tc.tile_pool(name="acts", bufs=8) as actp,
            tc.tile_pool(name="osb", bufs=4) as osbp,
            tc.tile_pool(name="psum", bufs=4, space="PSUM") as psp,
        ):
            w0_t = consts.tile([128, E, H], MMDT)
            nc.sync.dma_start(out=w0_t[:], in_=w0[:])
            w1_t = consts.tile([128, E, 2, H], MMDT)
            nc.sync.dma_start(out=w1_t[:], in_=w1[:])
            w2_t = consts.tile([128, E, 2, H], MMDT)
            nc.sync.dma_start(out=w2_t[:], in_=w2[:])
            wo_t = consts.tile([128, E, 2], MMDT)
            nc.sync.dma_start(out=wo_t[:], in_=wo[:])
            b0_t = consts.tile([128, E, 2], F32)
            nc.sync.dma_start(out=b0_t[:], in_=b0[:])
            b1_t = consts.tile([128, E, 2], F32)
            nc.sync.dma_start(out=b1_t[:], in_=b1[:])
            b2_t = consts.tile([128, E, 2], F32)
            nc.sync.dma_start(out=b2_t[:], in_=b2[:])

            # flat unit list: (e, c0, w), repeated `reps` times
            units = []
            for _rep in range(reps):
                for e in range(E):
                    units.extend((e, c0, w) for c0, w in spans)
            n_units = len(units)

            # per-unit pipeline state
            xs = [None] * n_units      # x input tile
            a_cur = [None] * n_units   # latest activation tile
            cols_of = [None] * n_units

            def s0_load(u):
                e, c0, w = units[u]
                x = xin.tile([128, SUPER], MMDT, tag="x", name=f"x{u}")
                nc.sync.dma_start(out=x[:, :w], in_=xt[e, :, c0:c0 + w])
                xs[u] = x
                cols_of[u] = [(cs, min(CHUNK, w - cs))
                              for cs in range(0, w, CHUNK)]

            def s1_layer0(u):
                e, c0, w = units[u]
                a0 = actp.tile([128, 2 * SUPER], MMDT, tag="a", name=f"a0_{u}")
                for ht in range(2):
                    ps = psp.tile([128, SUPER], F32, tag="ps", name=f"ps0_{u}_{ht}")
                    for cs, cw in cols_of[u]:
                        nc.tensor.matmul(
                            ps[:, cs:cs + cw],
                            w0_t[:, e, ht * 128:(ht + 1) * 128],
                            xs[u][:, cs:cs + cw],
                        )
                    nc.scalar.activation(
                        out=a0[:, ht * w: ht * w + w],
                        in_=ps[:, :w],
                        func=Tanh,
                        bias=b0_t[:, e, ht:ht + 1],
                        scale=1.0,
                    )
                xs[u] = None
                a_cur[u] = a0

            def mid_layer(u, w_t, b_t, li):
                e, c0, w = units[u]
                prev = a_cur[u]
                a = actp.tile([128, 2 * SUPER], MMDT, tag="a", name=f"a{li}_{u}")
                for ht in range(2):
                    ps = psp.tile([128, SUPER], F32, tag="ps",
                                  name=f"ps{li}_{u}_{ht}")
                    for cs, cw in cols_of[u]:
                        for kt in range(2):
                            nc.tensor.matmul(
                                ps[:, cs:cs + cw],
                                w_t[:, e, kt, ht * 128:(ht + 1) * 128],
                                prev[:, kt * w + cs: kt * w + cs + cw],
                                start=(kt == 0),
                                stop=(kt == 1),
                            )
                    nc.scalar.activation(
                        out=a[:, ht * w: ht * w + w],
                        in_=ps[:, :w],
                        func=Tanh,
                        bias=b_t[:, e, ht:ht + 1],
                        scale=1.0,
                    )
                a_cur[u] = a

            def s2_layer1(u):
                mid_layer(u, w1_t, b1_t, 1)

            def s3_layer2(u):
                mid_layer(u, w2_t, b2_t, 2)

            def s4_out(u):
                e, c0, w = units[u]
                prev = a_cur[u]
                pso = psp.tile([128, SUPER], F32, tag="ps", name=f"pso_{u}")
                for cs, cw in cols_of[u]:
                    for kt in range(2):
                        nc.tensor.matmul(
                            pso[:1, cs:cs + cw],
                            wo_t[:, e, kt:kt + 1],
                            prev[:, kt * w + cs: kt * w + cs + cw],
                            start=(kt == 0),
                            stop=(kt == 1),
                        )
                a_cur[u] = None
                o_sb = osbp.tile([1, SUPER], F32, tag="osb", name=f"osb{u}")
                nc.vector.tensor_copy(out=o_sb[:1, :w], in_=pso[:1, :w])
                nc.sync.dma_start(out=out[e:e + 1, c0:c0 + w], in_=o_sb[:1, :w])

            # software-pipelined emission, skew 1 between stages:
            #   step t emits L0(t), L1(t-1), L2(t-2), out(t-3), x-load(t+1)
            s0_load(0)
            for t in range(n_units + 3):
                if t + 1 < n_units:
                    s0_load(t + 1)
                if t < n_units:
                    s1_layer0(t)
                if 0 <= t - 1 < n_units:
                    s2_layer1(t - 1)
                if 0 <= t - 2 < n_units:
                    s3_layer2(t - 2)
                if 0 <= t - 3 < n_units:
                    s4_out(t - 3)

    nc.compile()
    return nc


def _get_compiled(mode=MODE):
    if mode not in _COMPILED:
        _COMPILED[mode] = _build_program(reps=1, mode=mode)
    return _COMPILED[mode]


def _prep_core_inputs(fps, W0, b0, W1, b1, W2, b2, Wout, mode=MODE):
    """Host-side shard + layout prep. Returns list of per-core input dicts."""
    f32 = np.float32
    mdt = _np_dtype(mode)

    def cvt(a):
        return np.ascontiguousarray(a).astype(mdt, copy=False)

    w0_dev = cvt(np.transpose(W0, (1, 0, 2)))
    w1_dev = cvt(W1.reshape(E, 2, 128, H).transpose(2, 0, 1, 3))
    w2_dev = cvt(W2.reshape(E, 2, 128, H).transpose(2, 0, 1, 3))
    wo_dev = cvt(Wout.reshape(E, 2, 128).transpose(2, 0, 1))
    b0_dev = np.ascontiguousarray(b0.reshape(E, 2, 128).transpose(2, 0, 1)).astype(f32)
    b1_dev = np.ascontiguousarray(b1.reshape(E, 2, 128).transpose(2, 0, 1)).astype(f32)
    b2_dev = np.ascontiguousarray(b2.reshape(E, 2, 128).transpose(2, 0, 1)).astype(f32)

    in_maps = []
    for c in range(N_CORES):
        xtc = np.zeros((E, D, MP), mdt)
        for e in range(E):
            xtc[e, :, :MPC] = fps[e][c * MPC:(c + 1) * MPC].T.astype(mdt, copy=False)
        in_maps.append({
            "xt": xtc,
            "w0": w0_dev, "w1": w1_dev, "w2": w2_dev, "wo": wo_dev,
            "b0": b0_dev, "b1": b1_dev, "b2": b2_dev,
        })
    return in_maps


def _route_outputs(flat_per_elem, elems, n_atoms):
    """Replicate reference routing: idx = concat(nonzero(elems==e, size=M))
    then segment_sum. nonzero(size=M) truncates or zero-pads."""
    out = np.zeros((n_atoms,), np.float32)
    for e in range(E):
        idx_e = np.nonzero(elems == e)[0]
        if idx_e.shape[0] >= M_TOTAL:
            idx_e = idx_e[:M_TOTAL]
        else:
            idx_e = np.concatenate(
                [idx_e, np.zeros(M_TOTAL - idx_e.shape[0], idx_e.dtype)])
        np.add.at(out, idx_e, flat_per_elem[e])
    return out


def kernel(fps_0, fps_1, fps_2, fps_3, W0, b0, W1, b1, W2, b2, Wout,
           elems, ind_1):
    from concourse.bass_utils import run_bass_kernel_spmd

    f32 = np.float32
    fps = [np.asarray(f, dtype=f32) for f in (fps_0, fps_1, fps_2, fps_3)]
    W0 = np.asarray(W0, dtype=f32)
    W1 = np.asarray(W1, dtype=f32)
    W2 = np.asarray(W2, dtype=f32)
    Wout = np.asarray(Wout, dtype=f32)
    b0 = np.asarray(b0, dtype=f32)
    b1 = np.asarray(b1, dtype=f32)
    b2 = np.asarray(b2, dtype=f32)
    elems = np.asarray(elems)
    n_atoms = np.asarray(ind_1).shape[0]

    nc = _get_compiled()
    in_maps = _prep_core_inputs(fps, W0, b0, W1, b1, W2, b2, Wout)
    res = run_bass_kernel_spmd(nc, in_maps, core_ids=list(range(N_CORES)))

    # [E, M_TOTAL] in element-major order (same as reference's out_e)
    flat = np.empty((E, M_TOTAL), f32)
    for c in range(N_CORES):
        o = res.results[c]["out"]          # [E, MP]
        flat[:, c * MPC:(c + 1) * MPC] = o[:, :MPC]

    out = _route_outputs(flat, elems, n_atoms)
    return out.reshape(n_atoms, 1).astype(f32)
